# revision 36
# baseline (speedup 1.0000x reference)
"""4D circular cross-correlation (qcd_ml C_Convolution, k=3, nd=4) on 8 TRN2 cores.

Math: out[o, x,y,z,t, s,c] = b[o] + sum_{i, ax,ay,az,at} W[i,o,ax,ay,az,at]
                                   * U[i, x+ax-1, y+ay-1, z+az-1, t+at-1, s,c]
(all site indices circular). U complex64 (4,16,16,16,32,4,3), W complex64
(4,4,3,3,3,3), b complex64 (4,).

Device mapping (per core, T sharded 8-way):
  - contraction (matmul partition) dim = (reim_in 2, C_in 4, X 16) = 128
  - output (PSUM partition) dim       = (reim_out 2, C_out 4, X0 16) = 128
  - X offsets (ax) live inside the stationary 128x128 matrices, circularly
    banded in (x, x0); complex arithmetic is the 2x2 [[Wr, Wi], [-Wi, Wr]]
    block over the reim axes.
  - T offsets (at) removed by a host-side Winograd F(4,3) transform along t
    (6 t-phases per 4 local t outputs).
  - Y offsets (ay) removed by a host-side Winograd F(2,3) transform along y
    (4 y-phases per y-pair): per (tph, yph) only the 3 az offsets remain as
    PSUM-accumulated matmuls.  PE work: 4 duos x 6 tph x 4 yph x 3 az = 288
    matmuls of 384 cols (vs 432 without the y transform).
  - moving free dim = (pair-in-duo 2, z 16, spin*color 12) = 384.
  - z circular handled by host padding to 18; t halo from neighbor T-slab;
    y halo inside the host B_y^T window.
  - The device applies only the cheap A_y^T combine (4 DVE ops per 3-slot
    piece, fp16 SBUF = 2x packed mode) and ships the SIX y-combined t-phase
    planes; the A_t^T inverse transform runs on the host in float64 during
    assembly.  This keeps total DVE work (~25us) and Act drain work (~40us,
    split with DVE) under the ~55us PE stream, which is the bottleneck.
  - PSUM->SBUF drains run on the Act engine only (~1.54us per t-phase,
    93% of the slot period but strictly FIFO with nothing else queued);
    DVE drains measured slower (fp32 src = 1x mode) and stalled the PE.
"""

import os
import sys
import itertools
import numpy as np

for _p in ("/opt/trn_rl_repo",):
    if _p not in sys.path and os.path.isdir(_p):
        sys.path.insert(0, _p)

C_IN, C_OUT = 4, 4
X = Y = Z = 16
T = 32
SC = 12  # spin*color
NCORES = 8
TLOC = T // NCORES          # 4 = one F(4,3) output tile
NPH = 6                     # Winograd F(4,3) t-phases
NYPH = 4                    # Winograd F(2,3) y-phases
NDUO = 4                    # 8 y-pairs in duos of 2
ZPAD = Z + 2                # 18
FREE = 2 * Z * SC           # 384: (pair-in-duo, z, sc)

# 16-bit data path: halves input DMA, halves LoadStationary (fp32r LS ~195ns
# would be the pipeline bottleneck; 16-bit LS ~97ns < 160ns matmul), and the
# fp16 SBUF y-combine runs the DVE at 2x packed mode. fp16 over bf16: same
# PE rate, 4x finer mantissa. Values are small (|U~|<~60, |wstat|<=4,
# |m|<~3000 < 65504) so fp16 range is safe.
CONV_DT = os.environ.get("CONV_DT", "fp16")

# Winograd F(4,3) along t, points [0,1,-1,2,-2,inf] (correlation form:
# out[r] = sum_k g[k] d[r+k], r=0..3, d = U[t0-1 .. t0+4]).
BT = np.array([
    [4, 0, -5, 0, 1, 0],
    [0, -4, -4, 1, 1, 0],
    [0, 4, -4, -1, 1, 0],
    [0, -2, -1, 2, 1, 0],
    [0, 2, -1, -2, 1, 0],
    [0, 4, 0, -5, 0, 1]], np.float64)
GT = np.array([
    [1 / 4, 0, 0],
    [-1 / 6, -1 / 6, -1 / 6],
    [-1 / 6, 1 / 6, -1 / 6],
    [1 / 24, 1 / 12, 1 / 6],
    [1 / 24, -1 / 12, 1 / 6],
    [0, 0, 1]], np.float64)
AT = np.array([
    [1, 1, 1, 1, 1, 0],
    [0, 1, -1, 2, -2, 0],
    [0, 1, 1, 4, 4, 0],
    [0, 1, -1, 8, -8, 1]], np.float64)   # applied on the HOST (assembly)

# Winograd F(2,3) along y, points [0,1,-1,inf] (correlation form:
# out[r] = sum_k g[k] d[r+k], r=0..1, d = U[y0-1 .. y0+2]).
BY = np.array([
    [1, 0, -1, 0],
    [0, 1, 1, 0],
    [0, -1, 1, 0],
    [0, 1, 0, -1]], np.float64)
GY = np.array([
    [1, 0, 0],
    [0.5, 0.5, 0.5],
    [0.5, -0.5, 0.5],
    [0, 0, 1]], np.float64)
# A_y^T = [[1,1,1,0],[0,1,-1,-1]] -- applied on the device (DVE).


def _np_dt():
    if CONV_DT == "fp16":
        return np.dtype(np.float16)
    if CONV_DT == "bf16":
        import ml_dtypes
        return np.dtype(ml_dtypes.bfloat16)
    return np.dtype(np.float32)


def _prep_u_shards(U):
    """U complex (4,16,16,16,32,4,3) -> per-core arrays
    [128, NDUO, NPH, NYPH, 2, ZPAD, SC] of the (t,y)-Winograd field.

    Layout is (duo, tph)-chunk-major so the DMA stream matches compute
    order: one contiguous chunk per (duo, t-phase)."""
    dt = _np_dt()
    Ur = np.stack([U.real, U.imag], axis=0).astype(np.float64)  # (2,4,X,Y,Z,T,4,3)
    Ur = Ur.reshape(2, C_IN, X, Y, Z, T, SC)
    Upz = np.pad(Ur, ((0, 0), (0, 0), (0, 0), (0, 0), (1, 1), (0, 0), (0, 0)),
                 mode="wrap")  # z -> 18
    # y windows per pair: rows (2p-1 .. 2p+2) mod 16
    yidx = (2 * np.arange(Y // 2)[:, None] - 1 + np.arange(4)[None, :]) % Y
    shards = []
    for k in range(NCORES):
        t0 = k * TLOC
        tidx = np.arange(t0 - 1, t0 + 5) % T        # 6-point t window
        d = np.take(Upz, tidx, axis=5)              # (2,4,16,16,18,6,12)
        mt = np.einsum("pk,rixyzks->rixyzps", BT, d)  # tph: (2,4,16,16,18,6,12)
        dy = mt[:, :, :, yidx]                      # (2,4,16, pair8, j4, 18, 6, 12)
        m = np.einsum("qj,rixpjzts->rixptqzs", BY, dy)
        # (2,4,16, pair8, tph6, yph4, z18, s12) -> split pair into (duo, ind)
        m = m.reshape(2, C_IN, X, NDUO, 2, NPH, NYPH, ZPAD, SC)
        m = m.transpose(0, 1, 2, 3, 5, 6, 4, 7, 8)  # (...,duo,tph,yph,ind,z,s)
        m = m.reshape(128, NDUO, NPH, NYPH, 2, ZPAD, SC).astype(dt)
        shards.append(np.ascontiguousarray(m))
    return shards


def _prep_wstat(W):
    """W complex (4,4,3,3,3,3) -> [128, NPH, NYPH, 3, 128] stationary stack.

    Wg[tph,yph][i,o,ax,az] = sum_{at,ay} GT[tph,at] GY[yph,ay] W[i,o,ax,ay,az,at];
    band in (x,x0): x = (x0 + ax - 1) mod 16;
    ri block M = [[Wr, Wi], [-Wi, Wr]] (columns riO: out_r, out_i).
    """
    Wc = np.ascontiguousarray(W).astype(np.complex128)
    Wg = np.einsum("pt,qy,ioxyzt->pqioxz", GT.astype(np.complex128),
                   GY.astype(np.complex128), Wc)   # (6,4,4,4,3,3)
    stat = np.zeros((2, C_IN, X, NPH, NYPH, 3, 2, C_OUT, X), _np_dt())
    for ph in range(NPH):
        for q in range(NYPH):
            for az in range(3):
                for ax in range(3):
                    wr = Wg[ph, q, :, :, ax, az].real.astype(np.float64)
                    wi = Wg[ph, q, :, :, ax, az].imag.astype(np.float64)
                    for x0 in range(X):
                        x = (x0 + ax - 1) % X
                        stat[0, :, x, ph, q, az, 0, :, x0] = wr
                        stat[1, :, x, ph, q, az, 0, :, x0] = -wi
                        stat[0, :, x, ph, q, az, 1, :, x0] = wi
                        stat[1, :, x, ph, q, az, 1, :, x0] = wr
    return np.ascontiguousarray(stat.reshape(128, NPH, NYPH, 3, 128))


def _assemble(results, b):
    """results[k]["out"]: [128, NDUO, NPH, 2, FREE] y-combined t-phase
    planes -> A_t^T on the host -> complex (4,16,16,16,32,4,3)."""
    out = np.empty((C_OUT, X, Y, Z, T, SC), np.complex64)
    for k in range(NCORES):
        yb = np.asarray(results[k]["out"], np.float32)  # (128, duo, tph, yrow, FREE)
        r = np.einsum("rp,KdpyF->KdryF", AT.astype(np.float32), yb)
        r = r.reshape(2, C_OUT, X, NDUO, TLOC, 2, 2, Z, SC)
        # axes: (ri, o, x, duo, t, yrow, ind, z, s); y = 4*duo + 2*ind + yrow
        r = r.transpose(0, 1, 2, 3, 6, 5, 7, 4, 8).reshape(
            2, C_OUT, X, Y, Z, TLOC, SC)
        out[:, :, :, :, k * TLOC:(k + 1) * TLOC, :] = r[0] + 1j * r[1]
    out += np.asarray(b, np.complex64).reshape(C_OUT, 1, 1, 1, 1, 1)
    return np.ascontiguousarray(out.reshape(C_OUT, X, Y, Z, T, 4, 3))


def _build_nc():
    import concourse.mybir as mybir
    from concourse import bacc, tile
    from contextlib import ExitStack

    f32 = mybir.dt.float32
    _dt16 = {"fp16": mybir.dt.float16, "bf16": mybir.dt.bfloat16}
    mm_dt = _dt16.get(CONV_DT, mybir.dt.float32r)
    out_dt = _dt16.get(CONV_DT, f32)
    cdt = out_dt  # combine dtype: 16-bit SBUF keeps the DVE in 2x mode

    nc = bacc.Bacc()
    w_dram = nc.declare_dram_parameter("wstat", [128, NPH, NYPH, 3, 128], mm_dt, isOutput=False)
    u_dram = nc.declare_dram_parameter("u", [128, NDUO, NPH, NYPH, 2, ZPAD, SC], mm_dt, isOutput=False)
    o_dram = nc.declare_dram_parameter("out", [128, NDUO, NPH, 2, FREE], out_dt, isOutput=True)

    with tile.TileContext(nc) as tc, ExitStack() as ctx:
        ipool = ctx.enter_context(tc.tile_pool(name="inp", bufs=1))
        mpool = ctx.enter_context(tc.tile_pool(name="mb", bufs=2))
        ypool = ctx.enter_context(tc.tile_pool(name="yc", bufs=2))
        ppool = ctx.enter_context(tc.tile_pool(name="psum", bufs=2, space="PSUM"))

        wt = ipool.tile([128, NPH, NYPH, 3, 128], mm_dt, tag="w")
        ufull = ipool.tile([128, NDUO, NPH, NYPH, 2, ZPAD, SC], mm_dt, tag="u")
        # Consumption-ordered input streaming on a single SP queue (aggregate
        # DMA bandwidth is shared across queues; explicit order beats
        # arbitration).  First matmul gate: wt[0,0] + u[0,0,0] ~ 0.2 MB.
        nc.sync.dma_start(wt[:, 0, 0:1], w_dram[:, 0, 0:1])
        nc.sync.dma_start(ufull[:, 0, 0, 0:1], u_dram[:, 0, 0, 0:1])
        nc.sync.dma_start(wt[:, 0, 1:4], w_dram[:, 0, 1:4])
        nc.sync.dma_start(ufull[:, 0, 0, 1:4], u_dram[:, 0, 0, 1:4])
        for ph in range(1, NPH):
            nc.sync.dma_start(wt[:, ph], w_dram[:, ph])
            nc.sync.dma_start(ufull[:, 0, ph], u_dram[:, 0, ph])
        for d in range(1, NDUO):
            for ph in range(NPH):
                nc.sync.dma_start(ufull[:, d, ph], u_dram[:, d, ph])

        # PE warm-up: dummy matmuls on a zeroed scratch tile while the input
        # DMA streams; keeps the PE HAM at full clock when real work starts.
        warm = ipool.tile([128, FREE], mm_dt, tag="warm")
        nc.gpsimd.memset(warm[:], 0.0)
        wps = ppool.tile([128, NYPH, 512], f32, tag="pt")
        for _ in range(4):
            # A short warm burst opens the HAM busy-window early; duo0's
            # first real matmuls (DMA-feed-paced anyway) finish warming it.
            nc.tensor.matmul(wps[:, 0, 0:FREE], warm[:, 0:128], warm[:],
                             start=True, stop=True)

        for d in range(NDUO):
            last = (d == NDUO - 1)
            mb = mpool.tile([128, NPH, NYPH, FREE], cdt, tag="mb")
            yb = ypool.tile([128, NPH, 2, FREE], cdt, tag="yb")
            s1 = ypool.tile([128, NPH, FREE], cdt, tag="s1", bufs=1)
            dd = ypool.tile([128, NPH, FREE], cdt, tag="dd", bufs=1)

            def ycomb(lo, hi):
                # A_y^T over drained slots [lo:hi): row0 = m0+m1+m2,
                # row1 = m1-m2-m3 (per y-pair), then ship the piece.
                nc.vector.tensor_add(s1[:, lo:hi], mb[:, lo:hi, 1, :], mb[:, lo:hi, 2, :])
                nc.vector.tensor_add(yb[:, lo:hi, 0, :], s1[:, lo:hi], mb[:, lo:hi, 0, :])
                nc.vector.tensor_sub(dd[:, lo:hi], mb[:, lo:hi, 1, :], mb[:, lo:hi, 2, :])
                nc.vector.tensor_sub(yb[:, lo:hi, 1, :], dd[:, lo:hi], mb[:, lo:hi, 3, :])
                # (out on the SP queue: SWDGE issue measured slower and the
                # epilogue waits for all transfers either way)
                nc.sync.dma_start(o_dram[:, d, lo:hi], yb[:, lo:hi])

            for ph in range(NPH):
                pt = ppool.tile([128, NYPH, 512], f32, tag="pt")
                for q in range(NYPH):
                    for az in range(3):
                        rhs = ufull[:, d, ph, q, :, az:az + Z, :]
                        nc.tensor.matmul(
                            pt[:, q, 0:FREE],
                            wt[:, ph, q, az, :],
                            rhs,
                            start=(az == 0),
                            stop=(az == 2),
                        )
                    if last and ph == 5 and q == 1:
                        nc.scalar.copy(mb[:, ph, 0:2], pt[:, 0:2, 0:FREE])
                # PSUM->SBUF drain (fp16) on the Act engine only: DVE
                # drains measured 1705ns (fp32 src forces 1x mode) and
                # queue behind y-combine ops, stalling PSUM recycling.
                if last and ph == 5:
                    # Split the final drain: the first half overlaps the
                    # last 6 matmuls (Act reads banks 0-1 while the PE
                    # writes banks 2-3), shortening the tail.
                    nc.scalar.copy(mb[:, ph, 2:4], pt[:, 2:4, 0:FREE])
                else:
                    nc.scalar.copy(mb[:, ph], pt[:, :, 0:FREE])
                if ph == 2:
                    # Ship the first half-duo early: with the single DMA
                    # queue, earlier out-chunks mean the queue drains
                    # sooner at kernel end (the exit epilogue waits on it).
                    ycomb(0, 3)
                elif last and ph == 4:
                    ycomb(3, 5)
            if last:
                ycomb(5, 6)
            else:
                ycomb(3, 6)

    # Bacc defers register allocation and sync-wait splitting to finalize();
    # run_bass_via_pjrt serializes the module as-is, so finalize here.
    nc.finalize()
    return nc


_NC_CACHE = None
LAST_RUN = None  # BassKernelResults of the most recent device run (for test.py)


def kernel(U, W, b):
    global _NC_CACHE, LAST_RUN
    shards = _prep_u_shards(np.asarray(U))
    wstat = _prep_wstat(np.asarray(W))

    if os.environ.get("CONV_EMULATE", "0") == "1":
        results = _emulate(shards, wstat)
    else:
        from concourse.bass_utils import run_bass_kernel_spmd
        if _NC_CACHE is None:
            _NC_CACHE = _build_nc()
        in_maps = [{"wstat": wstat, "u": u} for u in shards]
        trace = os.environ.get("CONV_TRACE", "0") == "1"
        LAST_RUN = run_bass_kernel_spmd(
            _NC_CACHE, in_maps, core_ids=list(range(NCORES)), trace=trace)
        results = LAST_RUN.results
    return _assemble(results, np.asarray(b))


def _emulate(shards, wstat):
    """Host-side emulation of the device program, mimicking the fp16
    rounding of the PSUM drain and each DVE op."""
    dt = _np_dt()

    def rnd(x):
        return x.astype(dt).astype(np.float64)

    results = []
    for u in shards:
        u = np.asarray(u, np.float64)
        w = np.asarray(wstat, np.float64)
        out = np.zeros((128, NDUO, NPH, 2, FREE), np.float64)
        for d in range(NDUO):
            mbuf = np.zeros((128, NPH, NYPH, FREE), np.float64)
            for ph in range(NPH):
                for q in range(NYPH):
                    acc = np.zeros((128, FREE), np.float64)
                    for az in range(3):
                        slab = u[:, d, ph, q, :, az:az + Z, :].reshape(128, -1)
                        acc += w[:, ph, q, az, :].T @ slab
                    mbuf[:, ph, q] = rnd(acc)
            # A_y^T
            s1 = rnd(mbuf[:, :, 1] + mbuf[:, :, 2])
            out[:, d, :, 0] = rnd(s1 + mbuf[:, :, 0])
            ddv = rnd(mbuf[:, :, 1] - mbuf[:, :, 2])
            out[:, d, :, 1] = rnd(ddv - mbuf[:, :, 3])
        results.append({"out": rnd(out)})
    return results


# revision 37
# speedup vs baseline: 1.0073x; 1.0073x over previous
"""4D circular cross-correlation (qcd_ml C_Convolution, k=3, nd=4) on 8 TRN2 cores.

Math: out[o, x,y,z,t, s,c] = b[o] + sum_{i, ax,ay,az,at} W[i,o,ax,ay,az,at]
                                   * U[i, x+ax-1, y+ay-1, z+az-1, t+at-1, s,c]
(all site indices circular). U complex64 (4,16,16,16,32,4,3), W complex64
(4,4,3,3,3,3), b complex64 (4,).

Device mapping (per core, T sharded 8-way):
  - contraction (matmul partition) dim = (reim_in 2, C_in 4, X 16) = 128
  - output (PSUM partition) dim       = (reim_out 2, C_out 4, X0 16) = 128
  - X offsets (ax) live inside the stationary 128x128 matrices, circularly
    banded in (x, x0); complex arithmetic is the 2x2 [[Wr, Wi], [-Wi, Wr]]
    block over the reim axes.
  - T offsets (at) removed by a host-side Winograd F(4,3) transform along t
    (6 t-phases per 4 local t outputs).
  - Y offsets (ay) removed by a host-side Winograd F(2,3) transform along y
    (4 y-phases per y-pair): per (tph, yph) only the 3 az offsets remain as
    PSUM-accumulated matmuls.  PE work: 4 duos x 6 tph x 4 yph x 3 az = 288
    matmuls of 384 cols (vs 432 without the y transform).
  - moving free dim = (pair-in-duo 2, z 16, spin*color 12) = 384.
  - z circular handled by host padding to 18; t halo from neighbor T-slab;
    y halo inside the host B_y^T window.
  - The device applies only the cheap A_y^T combine (4 DVE ops per 3-slot
    piece, fp16 SBUF = 2x packed mode) and ships the SIX y-combined t-phase
    planes; the A_t^T inverse transform runs on the host in float64 during
    assembly.  This keeps total DVE work (~25us) and Act drain work (~40us,
    split with DVE) under the ~55us PE stream, which is the bottleneck.
  - PSUM->SBUF drains run on the Act engine only (~1.54us per t-phase,
    93% of the slot period but strictly FIFO with nothing else queued);
    DVE drains measured slower (fp32 src = 1x mode) and stalled the PE.
"""

import os
import sys
import itertools
import numpy as np

for _p in ("/opt/trn_rl_repo",):
    if _p not in sys.path and os.path.isdir(_p):
        sys.path.insert(0, _p)

C_IN, C_OUT = 4, 4
X = Y = Z = 16
T = 32
SC = 12  # spin*color
NCORES = 8
TLOC = T // NCORES          # 4 = one F(4,3) output tile
NPH = 6                     # Winograd F(4,3) t-phases
NYPH = 4                    # Winograd F(2,3) y-phases
NDUO = 4                    # 8 y-pairs in duos of 2
ZPAD = Z + 2                # 18
FREE = 2 * Z * SC           # 384: (pair-in-duo, z, sc)

# 16-bit data path: halves input DMA, halves LoadStationary (fp32r LS ~195ns
# would be the pipeline bottleneck; 16-bit LS ~97ns < 160ns matmul), and the
# fp16 SBUF y-combine runs the DVE at 2x packed mode. fp16 over bf16: same
# PE rate, 4x finer mantissa. Values are small (|U~|<~60, |wstat|<=4,
# |m|<~3000 < 65504) so fp16 range is safe.
CONV_DT = os.environ.get("CONV_DT", "fp16")

# Winograd F(4,3) along t, points [0,1,-1,2,-2,inf] (correlation form:
# out[r] = sum_k g[k] d[r+k], r=0..3, d = U[t0-1 .. t0+4]).
BT = np.array([
    [4, 0, -5, 0, 1, 0],
    [0, -4, -4, 1, 1, 0],
    [0, 4, -4, -1, 1, 0],
    [0, -2, -1, 2, 1, 0],
    [0, 2, -1, -2, 1, 0],
    [0, 4, 0, -5, 0, 1]], np.float64)
GT = np.array([
    [1 / 4, 0, 0],
    [-1 / 6, -1 / 6, -1 / 6],
    [-1 / 6, 1 / 6, -1 / 6],
    [1 / 24, 1 / 12, 1 / 6],
    [1 / 24, -1 / 12, 1 / 6],
    [0, 0, 1]], np.float64)
AT = np.array([
    [1, 1, 1, 1, 1, 0],
    [0, 1, -1, 2, -2, 0],
    [0, 1, 1, 4, 4, 0],
    [0, 1, -1, 8, -8, 1]], np.float64)   # applied on the HOST (assembly)

# Winograd F(2,3) along y, points [0,1,-1,inf] (correlation form:
# out[r] = sum_k g[k] d[r+k], r=0..1, d = U[y0-1 .. y0+2]).
BY = np.array([
    [1, 0, -1, 0],
    [0, 1, 1, 0],
    [0, -1, 1, 0],
    [0, 1, 0, -1]], np.float64)
GY = np.array([
    [1, 0, 0],
    [0.5, 0.5, 0.5],
    [0.5, -0.5, 0.5],
    [0, 0, 1]], np.float64)
# A_y^T = [[1,1,1,0],[0,1,-1,-1]] -- applied on the device (DVE).


def _np_dt():
    if CONV_DT == "fp16":
        return np.dtype(np.float16)
    if CONV_DT == "bf16":
        import ml_dtypes
        return np.dtype(ml_dtypes.bfloat16)
    return np.dtype(np.float32)


def _prep_u_shards(U):
    """U complex (4,16,16,16,32,4,3) -> per-core arrays
    [128, NDUO, NPH, NYPH, 2, ZPAD, SC] of the (t,y)-Winograd field.

    Layout is (duo, tph)-chunk-major so the DMA stream matches compute
    order: one contiguous chunk per (duo, t-phase)."""
    dt = _np_dt()
    Ur = np.stack([U.real, U.imag], axis=0).astype(np.float64)  # (2,4,X,Y,Z,T,4,3)
    Ur = Ur.reshape(2, C_IN, X, Y, Z, T, SC)
    Upz = np.pad(Ur, ((0, 0), (0, 0), (0, 0), (0, 0), (1, 1), (0, 0), (0, 0)),
                 mode="wrap")  # z -> 18
    # y windows per pair: rows (2p-1 .. 2p+2) mod 16
    yidx = (2 * np.arange(Y // 2)[:, None] - 1 + np.arange(4)[None, :]) % Y
    shards = []
    for k in range(NCORES):
        t0 = k * TLOC
        tidx = np.arange(t0 - 1, t0 + 5) % T        # 6-point t window
        d = np.take(Upz, tidx, axis=5)              # (2,4,16,16,18,6,12)
        mt = np.einsum("pk,rixyzks->rixyzps", BT, d)  # tph: (2,4,16,16,18,6,12)
        dy = mt[:, :, :, yidx]                      # (2,4,16, pair8, j4, 18, 6, 12)
        m = np.einsum("qj,rixpjzts->rixptqzs", BY, dy)
        # (2,4,16, pair8, tph6, yph4, z18, s12) -> split pair into (duo, ind)
        m = m.reshape(2, C_IN, X, NDUO, 2, NPH, NYPH, ZPAD, SC)
        m = m.transpose(0, 1, 2, 3, 5, 6, 4, 7, 8)  # (...,duo,tph,yph,ind,z,s)
        m = m.reshape(128, NDUO, NPH, NYPH, 2, ZPAD, SC).astype(dt)
        shards.append(np.ascontiguousarray(m))
    return shards


def _prep_wstat(W):
    """W complex (4,4,3,3,3,3) -> [128, NPH, NYPH, 3, 128] stationary stack.

    Wg[tph,yph][i,o,ax,az] = sum_{at,ay} GT[tph,at] GY[yph,ay] W[i,o,ax,ay,az,at];
    band in (x,x0): x = (x0 + ax - 1) mod 16;
    ri block M = [[Wr, Wi], [-Wi, Wr]] (columns riO: out_r, out_i).
    """
    Wc = np.ascontiguousarray(W).astype(np.complex128)
    Wg = np.einsum("pt,qy,ioxyzt->pqioxz", GT.astype(np.complex128),
                   GY.astype(np.complex128), Wc)   # (6,4,4,4,3,3)
    stat = np.zeros((2, C_IN, X, NPH, NYPH, 3, 2, C_OUT, X), _np_dt())
    for ph in range(NPH):
        for q in range(NYPH):
            for az in range(3):
                for ax in range(3):
                    wr = Wg[ph, q, :, :, ax, az].real.astype(np.float64)
                    wi = Wg[ph, q, :, :, ax, az].imag.astype(np.float64)
                    for x0 in range(X):
                        x = (x0 + ax - 1) % X
                        stat[0, :, x, ph, q, az, 0, :, x0] = wr
                        stat[1, :, x, ph, q, az, 0, :, x0] = -wi
                        stat[0, :, x, ph, q, az, 1, :, x0] = wi
                        stat[1, :, x, ph, q, az, 1, :, x0] = wr
    return np.ascontiguousarray(stat.reshape(128, NPH, NYPH, 3, 128))


def _assemble(results, b):
    """results[k]["out"]: [128, NDUO, NPH, 2, FREE] y-combined t-phase
    planes -> A_t^T on the host -> complex (4,16,16,16,32,4,3)."""
    out = np.empty((C_OUT, X, Y, Z, T, SC), np.complex64)
    for k in range(NCORES):
        yb = np.asarray(results[k]["out"], np.float32)  # (128, duo, tph, yrow, FREE)
        r = np.einsum("rp,KdpyF->KdryF", AT.astype(np.float32), yb)
        r = r.reshape(2, C_OUT, X, NDUO, TLOC, 2, 2, Z, SC)
        # axes: (ri, o, x, duo, t, yrow, ind, z, s); y = 4*duo + 2*ind + yrow
        r = r.transpose(0, 1, 2, 3, 6, 5, 7, 4, 8).reshape(
            2, C_OUT, X, Y, Z, TLOC, SC)
        out[:, :, :, :, k * TLOC:(k + 1) * TLOC, :] = r[0] + 1j * r[1]
    out += np.asarray(b, np.complex64).reshape(C_OUT, 1, 1, 1, 1, 1)
    return np.ascontiguousarray(out.reshape(C_OUT, X, Y, Z, T, 4, 3))


def _build_nc():
    import concourse.mybir as mybir
    from concourse import bacc, tile
    from contextlib import ExitStack

    f32 = mybir.dt.float32
    _dt16 = {"fp16": mybir.dt.float16, "bf16": mybir.dt.bfloat16}
    mm_dt = _dt16.get(CONV_DT, mybir.dt.float32r)
    out_dt = _dt16.get(CONV_DT, f32)
    cdt = out_dt  # combine dtype: 16-bit SBUF keeps the DVE in 2x mode

    nc = bacc.Bacc()
    w_dram = nc.declare_dram_parameter("wstat", [128, NPH, NYPH, 3, 128], mm_dt, isOutput=False)
    u_dram = nc.declare_dram_parameter("u", [128, NDUO, NPH, NYPH, 2, ZPAD, SC], mm_dt, isOutput=False)
    o_dram = nc.declare_dram_parameter("out", [128, NDUO, NPH, 2, FREE], out_dt, isOutput=True)

    with tile.TileContext(nc) as tc, ExitStack() as ctx:
        ipool = ctx.enter_context(tc.tile_pool(name="inp", bufs=1))
        mpool = ctx.enter_context(tc.tile_pool(name="mb", bufs=2))
        ypool = ctx.enter_context(tc.tile_pool(name="yc", bufs=2))
        ppool = ctx.enter_context(tc.tile_pool(name="psum", bufs=2, space="PSUM"))

        wt = ipool.tile([128, NPH, NYPH, 3, 128], mm_dt, tag="w")
        ufull = ipool.tile([128, NDUO, NPH, NYPH, 2, ZPAD, SC], mm_dt, tag="u")
        # Consumption-ordered input streaming on a single SP queue (aggregate
        # DMA bandwidth is shared across queues; explicit order beats
        # arbitration).  First matmul gate: wt[0,0] + u[0,0,0] ~ 0.2 MB.
        nc.sync.dma_start(wt[:, 0, 0:1], w_dram[:, 0, 0:1])
        nc.sync.dma_start(ufull[:, 0, 0, 0:1], u_dram[:, 0, 0, 0:1])
        nc.sync.dma_start(wt[:, 0, 1:4], w_dram[:, 0, 1:4])
        nc.sync.dma_start(ufull[:, 0, 0, 1:4], u_dram[:, 0, 0, 1:4])
        for ph in range(1, NPH):
            nc.sync.dma_start(wt[:, ph], w_dram[:, ph])
            nc.sync.dma_start(ufull[:, 0, ph], u_dram[:, 0, ph])
        for d in range(1, NDUO):
            for ph in range(NPH):
                nc.sync.dma_start(ufull[:, d, ph], u_dram[:, d, ph])

        # PE warm-up: dummy matmuls on a zeroed scratch tile while the input
        # DMA streams; keeps the PE HAM at full clock when real work starts.
        warm = ipool.tile([128, FREE], mm_dt, tag="warm")
        nc.gpsimd.memset(warm[:], 0.0)
        wps = ppool.tile([128, NYPH, 512], f32, tag="pt")
        for _ in range(15):
            # ~15 x 320ns (cold clock) covers the ~3.4us HAM busy-window
            # for full PE clock AND bridges the input-DMA gate (fewer warm
            # matmuls measured worse: the real stream just runs cold and
            # stalls on the DMA feed instead).
            nc.tensor.matmul(wps[:, 0, 0:FREE], warm[:, 0:128], warm[:],
                             start=True, stop=True)

        for d in range(NDUO):
            last = (d == NDUO - 1)
            mb = mpool.tile([128, NPH, NYPH, FREE], cdt, tag="mb")
            yb = ypool.tile([128, NPH, 2, FREE], cdt, tag="yb")
            s1 = ypool.tile([128, NPH, FREE], cdt, tag="s1", bufs=1)
            dd = ypool.tile([128, NPH, FREE], cdt, tag="dd", bufs=1)

            def ycomb(lo, hi):
                # A_y^T over drained slots [lo:hi): row0 = m0+m1+m2,
                # row1 = m1-m2-m3 (per y-pair), then ship the piece.
                nc.vector.tensor_add(s1[:, lo:hi], mb[:, lo:hi, 1, :], mb[:, lo:hi, 2, :])
                nc.vector.tensor_add(yb[:, lo:hi, 0, :], s1[:, lo:hi], mb[:, lo:hi, 0, :])
                nc.vector.tensor_sub(dd[:, lo:hi], mb[:, lo:hi, 1, :], mb[:, lo:hi, 2, :])
                nc.vector.tensor_sub(yb[:, lo:hi, 1, :], dd[:, lo:hi], mb[:, lo:hi, 3, :])
                # (out on the SP queue: SWDGE issue measured slower and the
                # epilogue waits for all transfers either way)
                nc.sync.dma_start(o_dram[:, d, lo:hi], yb[:, lo:hi])

            for ph in range(NPH):
                pt = ppool.tile([128, NYPH, 512], f32, tag="pt")
                for q in range(NYPH):
                    for az in range(3):
                        rhs = ufull[:, d, ph, q, :, az:az + Z, :]
                        nc.tensor.matmul(
                            pt[:, q, 0:FREE],
                            wt[:, ph, q, az, :],
                            rhs,
                            start=(az == 0),
                            stop=(az == 2),
                        )
                # PSUM->SBUF drain (fp16) on the Act engine only: DVE
                # drains measured 1705ns (fp32 src forces 1x mode) and
                # queue behind y-combine ops, stalling PSUM recycling.
                nc.scalar.copy(mb[:, ph], pt[:, :, 0:FREE])
                if ph == 2:
                    # Ship the first half-duo early: with the single DMA
                    # queue, earlier out-chunks mean the queue drains
                    # sooner at kernel end (the exit epilogue waits on it).
                    ycomb(0, 3)
                elif last and ph == 4:
                    ycomb(3, 5)
            if last:
                ycomb(5, 6)
            else:
                ycomb(3, 6)

    # Bacc defers register allocation and sync-wait splitting to finalize();
    # run_bass_via_pjrt serializes the module as-is, so finalize here.
    nc.finalize()
    return nc


_NC_CACHE = None
LAST_RUN = None  # BassKernelResults of the most recent device run (for test.py)


def kernel(U, W, b):
    global _NC_CACHE, LAST_RUN
    shards = _prep_u_shards(np.asarray(U))
    wstat = _prep_wstat(np.asarray(W))

    if os.environ.get("CONV_EMULATE", "0") == "1":
        results = _emulate(shards, wstat)
    else:
        from concourse.bass_utils import run_bass_kernel_spmd
        if _NC_CACHE is None:
            _NC_CACHE = _build_nc()
        in_maps = [{"wstat": wstat, "u": u} for u in shards]
        trace = os.environ.get("CONV_TRACE", "0") == "1"
        LAST_RUN = run_bass_kernel_spmd(
            _NC_CACHE, in_maps, core_ids=list(range(NCORES)), trace=trace)
        results = LAST_RUN.results
    return _assemble(results, np.asarray(b))


def _emulate(shards, wstat):
    """Host-side emulation of the device program, mimicking the fp16
    rounding of the PSUM drain and each DVE op."""
    dt = _np_dt()

    def rnd(x):
        return x.astype(dt).astype(np.float64)

    results = []
    for u in shards:
        u = np.asarray(u, np.float64)
        w = np.asarray(wstat, np.float64)
        out = np.zeros((128, NDUO, NPH, 2, FREE), np.float64)
        for d in range(NDUO):
            mbuf = np.zeros((128, NPH, NYPH, FREE), np.float64)
            for ph in range(NPH):
                for q in range(NYPH):
                    acc = np.zeros((128, FREE), np.float64)
                    for az in range(3):
                        slab = u[:, d, ph, q, :, az:az + Z, :].reshape(128, -1)
                        acc += w[:, ph, q, az, :].T @ slab
                    mbuf[:, ph, q] = rnd(acc)
            # A_y^T
            s1 = rnd(mbuf[:, :, 1] + mbuf[:, :, 2])
            out[:, d, :, 0] = rnd(s1 + mbuf[:, :, 0])
            ddv = rnd(mbuf[:, :, 1] - mbuf[:, :, 2])
            out[:, d, :, 1] = rnd(ddv - mbuf[:, :, 3])
        results.append({"out": rnd(out)})
    return results


# revision 38
# speedup vs baseline: 1.0147x; 1.0074x over previous
"""4D circular cross-correlation (qcd_ml C_Convolution, k=3, nd=4) on 8 TRN2 cores.

Math: out[o, x,y,z,t, s,c] = b[o] + sum_{i, ax,ay,az,at} W[i,o,ax,ay,az,at]
                                   * U[i, x+ax-1, y+ay-1, z+az-1, t+at-1, s,c]
(all site indices circular). U complex64 (4,16,16,16,32,4,3), W complex64
(4,4,3,3,3,3), b complex64 (4,).

Device mapping (per core, T sharded 8-way):
  - contraction (matmul partition) dim = (reim_in 2, C_in 4, X 16) = 128
  - output (PSUM partition) dim       = (reim_out 2, C_out 4, X0 16) = 128
  - X offsets (ax) live inside the stationary 128x128 matrices, circularly
    banded in (x, x0); complex arithmetic is the 2x2 [[Wr, Wi], [-Wi, Wr]]
    block over the reim axes.
  - T offsets (at) removed by a host-side Winograd F(4,3) transform along t
    (6 t-phases per 4 local t outputs).
  - Y offsets (ay) removed by a host-side Winograd F(2,3) transform along y
    (4 y-phases per y-pair): per (tph, yph) only the 3 az offsets remain as
    PSUM-accumulated matmuls.  PE work: 4 duos x 6 tph x 4 yph x 3 az = 288
    matmuls of 384 cols (vs 432 without the y transform).
  - moving free dim = (pair-in-duo 2, z 16, spin*color 12) = 384.
  - z circular handled by host padding to 18; t halo from neighbor T-slab;
    y halo inside the host B_y^T window.
  - The device applies only the cheap A_y^T combine (4 DVE ops per 3-slot
    piece, fp16 SBUF = 2x packed mode) and ships the SIX y-combined t-phase
    planes; the A_t^T inverse transform runs on the host in float64 during
    assembly.  This keeps total DVE work (~25us) and Act drain work (~40us,
    split with DVE) under the ~55us PE stream, which is the bottleneck.
  - PSUM->SBUF drains run on the Act engine only (~1.54us per t-phase,
    93% of the slot period but strictly FIFO with nothing else queued);
    DVE drains measured slower (fp32 src = 1x mode) and stalled the PE.
"""

import os
import sys
import itertools
import numpy as np

for _p in ("/opt/trn_rl_repo",):
    if _p not in sys.path and os.path.isdir(_p):
        sys.path.insert(0, _p)

C_IN, C_OUT = 4, 4
X = Y = Z = 16
T = 32
SC = 12  # spin*color
NCORES = 8
TLOC = T // NCORES          # 4 = one F(4,3) output tile
NPH = 6                     # Winograd F(4,3) t-phases
NYPH = 4                    # Winograd F(2,3) y-phases
NDUO = 4                    # 8 y-pairs in duos of 2
ZPAD = Z + 2                # 18
FREE = 2 * Z * SC           # 384: (pair-in-duo, z, sc)

# 16-bit data path: halves input DMA, halves LoadStationary (fp32r LS ~195ns
# would be the pipeline bottleneck; 16-bit LS ~97ns < 160ns matmul), and the
# fp16 SBUF y-combine runs the DVE at 2x packed mode. fp16 over bf16: same
# PE rate, 4x finer mantissa. Values are small (|U~|<~60, |wstat|<=4,
# |m|<~3000 < 65504) so fp16 range is safe.
CONV_DT = os.environ.get("CONV_DT", "fp16")

# Winograd F(4,3) along t, points [0,1,-1,2,-2,inf] (correlation form:
# out[r] = sum_k g[k] d[r+k], r=0..3, d = U[t0-1 .. t0+4]).
BT = np.array([
    [4, 0, -5, 0, 1, 0],
    [0, -4, -4, 1, 1, 0],
    [0, 4, -4, -1, 1, 0],
    [0, -2, -1, 2, 1, 0],
    [0, 2, -1, -2, 1, 0],
    [0, 4, 0, -5, 0, 1]], np.float64)
GT = np.array([
    [1 / 4, 0, 0],
    [-1 / 6, -1 / 6, -1 / 6],
    [-1 / 6, 1 / 6, -1 / 6],
    [1 / 24, 1 / 12, 1 / 6],
    [1 / 24, -1 / 12, 1 / 6],
    [0, 0, 1]], np.float64)
AT = np.array([
    [1, 1, 1, 1, 1, 0],
    [0, 1, -1, 2, -2, 0],
    [0, 1, 1, 4, 4, 0],
    [0, 1, -1, 8, -8, 1]], np.float64)   # applied on the HOST (assembly)

# Winograd F(2,3) along y, points [0,1,-1,inf] (correlation form:
# out[r] = sum_k g[k] d[r+k], r=0..1, d = U[y0-1 .. y0+2]).
BY = np.array([
    [1, 0, -1, 0],
    [0, 1, 1, 0],
    [0, -1, 1, 0],
    [0, 1, 0, -1]], np.float64)
GY = np.array([
    [1, 0, 0],
    [0.5, 0.5, 0.5],
    [0.5, -0.5, 0.5],
    [0, 0, 1]], np.float64)
# A_y^T = [[1,1,1,0],[0,1,-1,-1]] -- applied on the device (DVE).


def _np_dt():
    if CONV_DT == "fp16":
        return np.dtype(np.float16)
    if CONV_DT == "bf16":
        import ml_dtypes
        return np.dtype(ml_dtypes.bfloat16)
    return np.dtype(np.float32)


def _prep_u_shards(U):
    """U complex (4,16,16,16,32,4,3) -> per-core arrays
    [128, NDUO, NPH, NYPH, 2, ZPAD, SC] of the (t,y)-Winograd field.

    Layout is (duo, tph)-chunk-major so the DMA stream matches compute
    order: one contiguous chunk per (duo, t-phase)."""
    dt = _np_dt()
    Ur = np.stack([U.real, U.imag], axis=0).astype(np.float64)  # (2,4,X,Y,Z,T,4,3)
    Ur = Ur.reshape(2, C_IN, X, Y, Z, T, SC)
    Upz = np.pad(Ur, ((0, 0), (0, 0), (0, 0), (0, 0), (1, 1), (0, 0), (0, 0)),
                 mode="wrap")  # z -> 18
    # y windows per pair: rows (2p-1 .. 2p+2) mod 16
    yidx = (2 * np.arange(Y // 2)[:, None] - 1 + np.arange(4)[None, :]) % Y
    shards = []
    for k in range(NCORES):
        t0 = k * TLOC
        tidx = np.arange(t0 - 1, t0 + 5) % T        # 6-point t window
        d = np.take(Upz, tidx, axis=5)              # (2,4,16,16,18,6,12)
        mt = np.einsum("pk,rixyzks->rixyzps", BT, d)  # tph: (2,4,16,16,18,6,12)
        dy = mt[:, :, :, yidx]                      # (2,4,16, pair8, j4, 18, 6, 12)
        m = np.einsum("qj,rixpjzts->rixptqzs", BY, dy)
        # (2,4,16, pair8, tph6, yph4, z18, s12) -> split pair into (duo, ind)
        m = m.reshape(2, C_IN, X, NDUO, 2, NPH, NYPH, ZPAD, SC)
        m = m.transpose(0, 1, 2, 3, 5, 6, 4, 7, 8)  # (...,duo,tph,yph,ind,z,s)
        m = m.reshape(128, NDUO, NPH, NYPH, 2, ZPAD, SC).astype(dt)
        shards.append(np.ascontiguousarray(m))
    return shards


def _prep_wstat(W):
    """W complex (4,4,3,3,3,3) -> [128, NPH, NYPH, 3, 128] stationary stack.

    Wg[tph,yph][i,o,ax,az] = sum_{at,ay} GT[tph,at] GY[yph,ay] W[i,o,ax,ay,az,at];
    band in (x,x0): x = (x0 + ax - 1) mod 16;
    ri block M = [[Wr, Wi], [-Wi, Wr]] (columns riO: out_r, out_i).
    """
    Wc = np.ascontiguousarray(W).astype(np.complex128)
    Wg = np.einsum("pt,qy,ioxyzt->pqioxz", GT.astype(np.complex128),
                   GY.astype(np.complex128), Wc)   # (6,4,4,4,3,3)
    stat = np.zeros((2, C_IN, X, NPH, NYPH, 3, 2, C_OUT, X), _np_dt())
    for ph in range(NPH):
        for q in range(NYPH):
            for az in range(3):
                for ax in range(3):
                    wr = Wg[ph, q, :, :, ax, az].real.astype(np.float64)
                    wi = Wg[ph, q, :, :, ax, az].imag.astype(np.float64)
                    for x0 in range(X):
                        x = (x0 + ax - 1) % X
                        stat[0, :, x, ph, q, az, 0, :, x0] = wr
                        stat[1, :, x, ph, q, az, 0, :, x0] = -wi
                        stat[0, :, x, ph, q, az, 1, :, x0] = wi
                        stat[1, :, x, ph, q, az, 1, :, x0] = wr
    return np.ascontiguousarray(stat.reshape(128, NPH, NYPH, 3, 128))


def _assemble(results, b):
    """results[k]["out"]: [128, NDUO, NPH, 2, FREE] y-combined t-phase
    planes -> A_t^T on the host -> complex (4,16,16,16,32,4,3)."""
    out = np.empty((C_OUT, X, Y, Z, T, SC), np.complex64)
    for k in range(NCORES):
        yb = np.asarray(results[k]["out"], np.float32)  # (128, duo, tph, yrow, FREE)
        r = np.einsum("rp,KdpyF->KdryF", AT.astype(np.float32), yb)
        r = r.reshape(2, C_OUT, X, NDUO, TLOC, 2, 2, Z, SC)
        # axes: (ri, o, x, duo, t, yrow, ind, z, s); y = 4*duo + 2*ind + yrow
        r = r.transpose(0, 1, 2, 3, 6, 5, 7, 4, 8).reshape(
            2, C_OUT, X, Y, Z, TLOC, SC)
        out[:, :, :, :, k * TLOC:(k + 1) * TLOC, :] = r[0] + 1j * r[1]
    out += np.asarray(b, np.complex64).reshape(C_OUT, 1, 1, 1, 1, 1)
    return np.ascontiguousarray(out.reshape(C_OUT, X, Y, Z, T, 4, 3))


def _build_nc():
    import concourse.mybir as mybir
    from concourse import bacc, tile
    from contextlib import ExitStack

    f32 = mybir.dt.float32
    _dt16 = {"fp16": mybir.dt.float16, "bf16": mybir.dt.bfloat16}
    mm_dt = _dt16.get(CONV_DT, mybir.dt.float32r)
    out_dt = _dt16.get(CONV_DT, f32)
    cdt = out_dt  # combine dtype: 16-bit SBUF keeps the DVE in 2x mode

    nc = bacc.Bacc()
    w_dram = nc.declare_dram_parameter("wstat", [128, NPH, NYPH, 3, 128], mm_dt, isOutput=False)
    u_dram = nc.declare_dram_parameter("u", [128, NDUO, NPH, NYPH, 2, ZPAD, SC], mm_dt, isOutput=False)
    o_dram = nc.declare_dram_parameter("out", [128, NDUO, NPH, 2, FREE], out_dt, isOutput=True)

    with tile.TileContext(nc) as tc, ExitStack() as ctx:
        ipool = ctx.enter_context(tc.tile_pool(name="inp", bufs=1))
        mpool = ctx.enter_context(tc.tile_pool(name="mb", bufs=2))
        ypool = ctx.enter_context(tc.tile_pool(name="yc", bufs=2))
        ppool = ctx.enter_context(tc.tile_pool(name="psum", bufs=2, space="PSUM"))

        wt = ipool.tile([128, NPH, NYPH, 3, 128], mm_dt, tag="w")
        ufull = ipool.tile([128, NDUO, NPH, NYPH, 2, ZPAD, SC], mm_dt, tag="u")
        # Consumption-ordered input streaming on a single SP queue (aggregate
        # DMA bandwidth is shared across queues; explicit order beats
        # arbitration).  First matmul gate: wt[0,0] + u[0,0,0] ~ 0.2 MB.
        nc.sync.dma_start(wt[:, 0, 0:1], w_dram[:, 0, 0:1])
        nc.sync.dma_start(ufull[:, 0, 0, 0:1], u_dram[:, 0, 0, 0:1])
        nc.sync.dma_start(wt[:, 0, 1:4], w_dram[:, 0, 1:4])
        nc.sync.dma_start(ufull[:, 0, 0, 1:4], u_dram[:, 0, 0, 1:4])
        nc.sync.dma_start(wt[:, 1], w_dram[:, 1])
        nc.sync.dma_start(ufull[:, 0, 1], u_dram[:, 0, 1])
        nc.sync.dma_start(wt[:, 2:4], w_dram[:, 2:4])
        nc.sync.dma_start(ufull[:, 0, 2:4], u_dram[:, 0, 2:4])
        nc.sync.dma_start(wt[:, 4:6], w_dram[:, 4:6])
        nc.sync.dma_start(ufull[:, 0, 4:6], u_dram[:, 0, 4:6])
        for d in range(1, NDUO):
            for ph in range(NPH):
                nc.sync.dma_start(ufull[:, d, ph], u_dram[:, d, ph])

        # PE warm-up: dummy matmuls on a zeroed scratch tile while the input
        # DMA streams; keeps the PE HAM at full clock when real work starts.
        warm = ipool.tile([128, FREE], mm_dt, tag="warm")
        nc.gpsimd.memset(warm[:], 0.0)
        wps = ppool.tile([128, NYPH, 512], f32, tag="pt")
        for _ in range(15):
            # ~15 x 320ns (cold clock) covers the ~3.4us HAM busy-window
            # for full PE clock AND bridges the input-DMA gate (fewer warm
            # matmuls measured worse: the real stream just runs cold and
            # stalls on the DMA feed instead).
            nc.tensor.matmul(wps[:, 0, 0:FREE], warm[:, 0:128], warm[:],
                             start=True, stop=True)

        for d in range(NDUO):
            last = (d == NDUO - 1)
            mb = mpool.tile([128, NPH, NYPH, FREE], cdt, tag="mb")
            yb = ypool.tile([128, NPH, 2, FREE], cdt, tag="yb")
            s1 = ypool.tile([128, NPH, FREE], cdt, tag="s1", bufs=1)
            dd = ypool.tile([128, NPH, FREE], cdt, tag="dd", bufs=1)

            def ycomb(lo, hi):
                # A_y^T over drained slots [lo:hi): row0 = m0+m1+m2,
                # row1 = m1-m2-m3 (per y-pair), then ship the piece.
                nc.vector.tensor_add(s1[:, lo:hi], mb[:, lo:hi, 1, :], mb[:, lo:hi, 2, :])
                nc.vector.tensor_add(yb[:, lo:hi, 0, :], s1[:, lo:hi], mb[:, lo:hi, 0, :])
                nc.vector.tensor_sub(dd[:, lo:hi], mb[:, lo:hi, 1, :], mb[:, lo:hi, 2, :])
                nc.vector.tensor_sub(yb[:, lo:hi, 1, :], dd[:, lo:hi], mb[:, lo:hi, 3, :])
                # (out on the SP queue: SWDGE issue measured slower and the
                # epilogue waits for all transfers either way)
                nc.sync.dma_start(o_dram[:, d, lo:hi], yb[:, lo:hi])

            for ph in range(NPH):
                pt = ppool.tile([128, NYPH, 512], f32, tag="pt")
                for q in range(NYPH):
                    for az in range(3):
                        rhs = ufull[:, d, ph, q, :, az:az + Z, :]
                        nc.tensor.matmul(
                            pt[:, q, 0:FREE],
                            wt[:, ph, q, az, :],
                            rhs,
                            start=(az == 0),
                            stop=(az == 2),
                        )
                # PSUM->SBUF drain (fp16) on the Act engine only: DVE
                # drains measured 1705ns (fp32 src forces 1x mode) and
                # queue behind y-combine ops, stalling PSUM recycling.
                nc.scalar.copy(mb[:, ph], pt[:, :, 0:FREE])
                if ph == 2:
                    # Ship the first half-duo early: with the single DMA
                    # queue, earlier out-chunks mean the queue drains
                    # sooner at kernel end (the exit epilogue waits on it).
                    ycomb(0, 3)
                elif last and ph == 3:
                    ycomb(3, 4)
                elif last and ph == 4:
                    ycomb(4, 5)
            if last:
                ycomb(5, 6)
            else:
                ycomb(3, 6)

    # Bacc defers register allocation and sync-wait splitting to finalize();
    # run_bass_via_pjrt serializes the module as-is, so finalize here.
    nc.finalize()
    return nc


_NC_CACHE = None
LAST_RUN = None  # BassKernelResults of the most recent device run (for test.py)


def kernel(U, W, b):
    global _NC_CACHE, LAST_RUN
    shards = _prep_u_shards(np.asarray(U))
    wstat = _prep_wstat(np.asarray(W))

    if os.environ.get("CONV_EMULATE", "0") == "1":
        results = _emulate(shards, wstat)
    else:
        from concourse.bass_utils import run_bass_kernel_spmd
        if _NC_CACHE is None:
            _NC_CACHE = _build_nc()
        in_maps = [{"wstat": wstat, "u": u} for u in shards]
        trace = os.environ.get("CONV_TRACE", "0") == "1"
        LAST_RUN = run_bass_kernel_spmd(
            _NC_CACHE, in_maps, core_ids=list(range(NCORES)), trace=trace)
        results = LAST_RUN.results
    return _assemble(results, np.asarray(b))


def _emulate(shards, wstat):
    """Host-side emulation of the device program, mimicking the fp16
    rounding of the PSUM drain and each DVE op."""
    dt = _np_dt()

    def rnd(x):
        return x.astype(dt).astype(np.float64)

    results = []
    for u in shards:
        u = np.asarray(u, np.float64)
        w = np.asarray(wstat, np.float64)
        out = np.zeros((128, NDUO, NPH, 2, FREE), np.float64)
        for d in range(NDUO):
            mbuf = np.zeros((128, NPH, NYPH, FREE), np.float64)
            for ph in range(NPH):
                for q in range(NYPH):
                    acc = np.zeros((128, FREE), np.float64)
                    for az in range(3):
                        slab = u[:, d, ph, q, :, az:az + Z, :].reshape(128, -1)
                        acc += w[:, ph, q, az, :].T @ slab
                    mbuf[:, ph, q] = rnd(acc)
            # A_y^T
            s1 = rnd(mbuf[:, :, 1] + mbuf[:, :, 2])
            out[:, d, :, 0] = rnd(s1 + mbuf[:, :, 0])
            ddv = rnd(mbuf[:, :, 1] - mbuf[:, :, 2])
            out[:, d, :, 1] = rnd(ddv - mbuf[:, :, 3])
        results.append({"out": rnd(out)})
    return results
